# revision 49
# baseline (speedup 1.0000x reference)
"""PeakDetectionLoss on 8 Trainium2 cores.

Sharding: time axis split into 8 segments (one per core), all 10 signal rows
(5 ppg + 5 rppg) on every core. Host pre-pads 5 samples of -inf at the global
edges and hands each core overlapping [128, 2058] windows per row, so the
width-11 sliding max needs no device halo exchange.

Per-row stats (peak count, signal sum, peak-value sum) are reduced across
partitions with a ones-matmul and across cores with two tiny AllReduces: the
one for rows 0-5 is issued mid-row-loop so its ~35us ring latency hides under
rows 6-9's compute, and the late one only gates the phase-2 work for rows
6-9. Slot/position extraction is deferred until after the collectives are
issued so it fills the remaining latency. Positions are strip-local fp16
(host adds global offsets; 24576 is the fp16-exact "no peak" sentinel, whose
reciprocal contribution to the gap sums is ~2e-5 per term and cancels between
the ppg/rppg rows). Row transfers are split across both HWDGE queues (one
sustains only ~75 GB/s), the ar DMAs ride the gpsimd SWDGE queue, and the
threshold partition-broadcast is a PE ones-matmul, so none of them
head-of-line-block the row DMA streams or the DVE. Host stitches the 1024
strip summaries per row (first/last peak, sum of 1/gap, weak count).
"""
import os
import sys

for _p in ("/opt/trn_rl_repo", "/root/.axon_site/_ro/trn_rl_repo"):
    if _p not in sys.path:
        sys.path.append(_p)

import numpy as np

N = 5
L = 2097152
C = 8
SEG = L // C            # 262144
P = 128
PW = SEG // P           # 2048
NB = PW // 4            # 512
TILE_W = PW + 10        # 2058
R = 2 * N               # 10 rows per core
BIG_H = 24576.0         # fp16-exact sentinel; 2*BIG_H stays finite in fp16
VALID_LT = 16384.0      # host-side validity threshold for positions
SEGW = 32               # segments per strip shipped to the host stitcher

_STATE = {}


def _build_program():
    from concourse import bacc, tile, mybir
    from concourse.alu_op_type import AluOpType as op

    f32 = mybir.dt.float32
    f16 = mybir.dt.float16
    i16 = mybir.dt.int16
    act = mybir.ActivationFunctionType
    nc = bacc.Bacc("TRN2", target_bir_lowering=False, debug=False, num_devices=C)

    xin = nc.dram_tensor("xin", [R, P, TILE_W], f32, kind="ExternalInput")
    summ = nc.dram_tensor("summ", [P, 40], f32, kind="ExternalOutput")
    summ2 = nc.dram_tensor("summ2", [P, 2 * R * SEGW], f16,
                           kind="ExternalOutput")

    with tile.TileContext(nc) as tc:
        with (
            tc.tile_pool(name="sb", bufs=1) as sb,
            tc.tile_pool(name="dram", bufs=1, space="DRAM") as dram,
            tc.tile_pool(name="ps", bufs=1, space="PSUM") as ps,
        ):
            ones = sb.tile([P, 1], f32, tag="ones")
            ones_row = sb.tile([1, P], f32, tag="ones_row")
            bigrow = sb.tile([P, NB], f16, tag="bigrow")
            # split stats tiles so the early AllReduce's matmul doesn't
            # falsely depend on rows 7-9's accumulates
            statsA = sb.tile([P, 3 * 6], f32, tag="statsA")
            statsB = sb.tile([P, 3 * (R - 6)], f32, tag="statsB")
            # bn (peak value per block) shares storage with the recip scratch
            # (bn is dead once B4m is built; recips are written later)
            bnscr = sb.tile([P, R * NB], f32, tag="bnscr")
            m1a = sb.tile([P, R * PW], f16, tag="m1a")
            n1a = sb.tile([P, R * NB], f16, tag="n1a")
            p1a = sb.tile([P, R * NB], f16, tag="p1a")
            p2a = sb.tile([P, R * NB], f16, tag="p2a")
            h = sb.tile([P, 2 * R * NB], f16, tag="h")
            # B4m holds the amplitude-test values; its first 2048 slots double
            # as the dummy activation output for the Sx accumulate
            B4m = sb.tile([P, R * NB], f32, tag="B4m")
            treeB = sb.tile([P, 2 * R * 256], f16, tag="treeB")
            treeC = sb.tile([P, 2 * R * 128], f16, tag="treeC")
            notvB = sb.tile([P, (R - 6) * NB], f16, tag="notvB")
            gapb = sb.tile([P, R * 256], f32, tag="gapb")
            r4 = sb.tile([P, NB], f16, tag="r4")
            tbc = sb.tile([P, R], f32, tag="tbc")
            arsbA = sb.tile([1, 3 * 6], f32, tag="arsbA")
            arsbB = sb.tile([1, 3 * (R - 6)], f32, tag="arsbB")
            arstA = sb.tile([1, 3 * 6], f32, tag="arstA")
            arstB = sb.tile([1, 3 * (R - 6)], f32, tag="arstB")
            trec = sb.tile([1, R], f32, tag="trec")
            tmean = sb.tile([1, R], f32, tag="tmean")
            tthr = sb.tile([1, R], f32, tag="tthr")
            summ_sb = sb.tile([P, 40], f32, tag="summ_sb")
            stg = sb.tile([P, 2 * R * SEGW], f16, tag="stg")
            # per-row chain scratch (statically reused; M8 aliases M2)
            M2 = sb.tile([P, 2057], f32, tag="M2")
            M4 = sb.tile([P, 2055], f32, tag="M4")
            Wt = sb.tile([P, PW], f32, tag="Wt")

            SPLIT = 18  # stats columns 0:18 (rows 0-5) reduce early
            ar_inA = dram.tile([1, SPLIT], f32)
            ar_outA = dram.tile([1, SPLIT], f32)
            ar_inB = dram.tile([1, 3 * R - SPLIT], f32)
            ar_outB = dram.tile([1, 3 * R - SPLIT], f32)
            psum_A = ps.tile([1, SPLIT], f32)
            psum_B = ps.tile([1, 3 * R - SPLIT], f32)
            psum_tbc = ps.tile([P, R], f32)

            xin_ap = xin.ap()
            nc.vector.memset(ones, 1.0)
            nc.vector.memset(ones_row, 1.0)
            nc.vector.memset(bigrow, BIG_H)
            nc.gpsimd.iota(r4, pattern=[[4, NB]], base=0, channel_multiplier=0,
                           allow_small_or_imprecise_dtypes=True)

            aBv = gapb.bitcast(f16)  # [P, 2*R*NB] fp16 view of the gap buffer
            m1v = m1a.rearrange("p (r n) -> p r n", r=R)
            n1v = n1a.rearrange("p (r n) -> p r n", r=R)
            p1v = p1a.rearrange("p (r n) -> p r n", r=R)
            p2v = p2a.rearrange("p (r n) -> p r n", r=R)
            bnv = bnscr.rearrange("p (r n) -> p r n", r=R)
            B4mv = B4m.rearrange("p (r n) -> p r n", r=R)

            for r in range(R):
                xt = sb.tile([P, TILE_W], f32, tag="xt", bufs=2, name=f"xt{r}")
                # each row's transfer split across both HWDGE queues
                # (one queue sustains only ~75 GB/s)
                nc.sync.dma_start(xt[:, 0:1029], xin_ap[r][:, 0:1029])
                nc.scalar.dma_start(xt[:, 1029:TILE_W], xin_ap[r][:, 1029:TILE_W])

                # sliding max chain (window 11, centered at xt[:, j+5])
                nc.vector.tensor_tensor(
                    out=M2, in0=xt[:, 0:2057], in1=xt[:, 1:2058], op=op.max)
                nc.vector.tensor_tensor(
                    out=M4, in0=M2[:, 0:2055], in1=M2[:, 2:2057], op=op.max)
                M8 = M2[:, 0:2051]  # M2 storage reused for M8
                nc.vector.tensor_tensor(
                    out=M8, in0=M4[:, 0:2051], in1=M4[:, 4:2055], op=op.max)
                nc.vector.tensor_tensor(
                    out=Wt, in0=M8[:, 0:PW], in1=M2[:, 3:3 + PW], op=op.max)

                stt = statsA if r < 6 else statsB
                sc = 3 * (r if r < 6 else r - 6)
                xc = xt[:, 5:5 + PW]
                # m1 = (x == window max); accum -> per-partition peak count
                nc.vector.scalar_tensor_tensor(
                    out=m1v[:, r], in0=xc, scalar=0.0, op0=op.bypass,
                    in1=Wt, op1=op.is_ge,
                    accum_out=stt[:, sc:sc + 1])
                # Sx on ScalarE (free accumulate; dummy out over B4m scratch)
                nc.scalar.activation(
                    out=B4m[:, 0:PW], in_=xc, func=act.Copy,
                    accum_out=stt[:, sc + 1:sc + 2])
                # peaks per aligned 4-block (0 or 1 each)
                with nc.allow_low_precision(reason="0/1 sums <=4, exact fp16"):
                    nc.vector.tensor_reduce(
                        out=n1v[:, r],
                        in_=m1v[:, r].rearrange("p (b k) -> p b k", k=4),
                        axis=mybir.AxisListType.X, op=op.add)
                B4 = M4[:, 5:2052:4]  # aligned block-4 max, free view of M4
                # bn = B4*n1 (fused accum -> sum of peak values)
                nc.vector.scalar_tensor_tensor(
                    out=bnv[:, r], in0=B4, scalar=0.0, op0=op.bypass,
                    in1=n1v[:, r], op1=op.mult,
                    accum_out=stt[:, sc + 2:sc + 3])

                if r == 5:
                    # aB-G1 on the idle Act engine while rows 6-9 compute
                    nc.scalar.activation(
                        out=aBv[:, 0:6 * NB], in_=n1a[:, 0:6 * NB],
                        func=act.Copy, bias=-BIG_H, scale=BIG_H)
                    # early AllReduce for rows 0-5; its ~35us ring latency
                    # hides under rows 6-9's compute. The psum->sbuf copy
                    # runs on DVE and the ar DMAs on the gpsimd SWDGE queue
                    # so neither blocks the xt row-DMA streams.
                    nc.tensor.matmul(
                        out=psum_A[0:1, :], lhsT=ones, rhs=statsA,
                        start=True, stop=True)
                    nc.vector.tensor_scalar(
                        out=arstA, in0=psum_A[0:1, :], scalar1=1.0,
                        scalar2=None, op0=op.mult)
                    nc.gpsimd.dma_start(ar_inA, arstA)
                    nc.gpsimd.collective_compute(
                        "AllReduce", op.add, replica_groups=[list(range(C))],
                        ins=[ar_inA.opt()], outs=[ar_outA.opt()])
                    nc.gpsimd.dma_start(arsbA, ar_outA)

            # deferred slot/position work overlaps the collective
            with nc.allow_low_precision(reason="slot sums <=3, exact fp16"):
                nc.vector.scalar_tensor_tensor(
                    out=p1v, in0=m1v[:, :, 2:PW:4], scalar=2.0, op0=op.mult,
                    in1=m1v[:, :, 1:PW:4], op1=op.add)
                nc.vector.scalar_tensor_tensor(
                    out=p2v, in0=m1v[:, :, 3:PW:4], scalar=3.0, op0=op.mult,
                    in1=p1v, op1=op.add)
            hpos = h[:, 0:R * NB]
            hneg = h[:, R * NB:2 * R * NB]
            with nc.allow_low_precision(reason="positions <=2047, exact fp16"):
                nc.vector.tensor_tensor(
                    out=hpos.rearrange("p (r n) -> p r n", r=R), in0=p2v,
                    in1=r4.unsqueeze(1).broadcast_to([P, R, NB]), op=op.add)
            nc.scalar.mul(hneg, hpos, -1.0)
            # aB-G2 = 0 if peak else -BIG_H (G1's ran mid-row-loop);
            # B4m = bn + aB per group (aB lives in gapb, dead before gaps)
            nc.scalar.activation(out=aBv[:, 6 * NB:R * NB],
                                 in_=n1a[:, 6 * NB:R * NB], func=act.Copy,
                                 bias=-BIG_H, scale=BIG_H)
            nc.vector.tensor_tensor(
                out=B4m[:, 0:6 * NB], in0=bnscr[:, 0:6 * NB],
                in1=aBv[:, 0:6 * NB], op=op.add)
            nc.vector.tensor_tensor(
                out=B4m[:, 6 * NB:R * NB], in0=bnscr[:, 6 * NB:R * NB],
                in1=aBv[:, 6 * NB:R * NB], op=op.add)

            # late AllReduce for rows 6-9, issued after the deferred DVE work
            # so its psum->sbuf copy can't head-of-line-block that work
            nc.tensor.matmul(
                out=psum_B[0:1, :], lhsT=ones, rhs=statsB,
                start=True, stop=True)
            nc.vector.tensor_scalar(
                out=arstB, in0=psum_B[0:1, :], scalar1=1.0,
                scalar2=None, op0=op.mult)
            nc.gpsimd.dma_start(ar_inB, arstB)
            nc.gpsimd.collective_compute(
                "AllReduce", op.add, replica_groups=[list(range(C))],
                ins=[ar_inB.opt()], outs=[ar_outB.opt()])
            nc.gpsimd.dma_start(arsbB, ar_outB)

            # phase 2 runs in two row groups: rows 0-6 only need the early
            # AllReduce, rows 7-9 wait for the late one
            h4 = h.rearrange("p (a r n) -> p a r n", a=2, r=R)
            treeB4 = treeB.rearrange("p (a r c) -> p a r c", a=2, r=R)
            treeC4 = treeC.rearrange("p (a r c) -> p a r c", a=2, r=R)
            gap3 = gapb.rearrange("p (r c) -> p r c", r=R)
            scr3 = bnscr.rearrange("p (r c) -> p r c", r=R)

            # G1's notv borrows treeB's first rows (dead before L1 lands there)
            # Low scheduler priority (negative offset = later) keeps these
            # collective-dependent ops from being hoisted into the row loop's
            # DVE stream, where their arsb wait would block it head-of-line.
            phase2 = tc.high_priority(offset=-1000000)
            phase2.__enter__()
            # WAW gate: the static scheduler underestimates the collective
            # latency and would slot the arsb-dependent threshold ops into
            # the DVE stream ahead of row 9, stalling it head-of-line. A
            # dummy write to trec that reads row 9's mask pins them after.
            nc.vector.tensor_scalar(
                out=trec[0:1, :], in0=m1a[0:1, 9 * PW:9 * PW + R],
                scalar1=0.0, scalar2=None, op0=op.mult)
            for lo, hi, ntile in ((0, 6, treeB[:, 0:6 * NB]), (6, R, notvB)):
                G = hi - lo
                arsb_g = arsbA if lo == 0 else arsbB
                # threshold t_r = Sx_r/(2L) + 0.5*sv_r/npk_r (raw-space)
                a_npk = arsb_g[0:1, 0:3 * G:3]
                a_sx = arsb_g[0:1, 1:3 * G:3]
                a_sv = arsb_g[0:1, 2:3 * G:3]
                nc.vector.reciprocal(out=trec[0:1, lo:hi], in_=a_npk)
                nc.vector.scalar_tensor_tensor(
                    out=tmean[0:1, lo:hi], in0=trec[0:1, lo:hi], scalar=0.5,
                    op0=op.mult, in1=a_sv, op1=op.mult)
                nc.vector.scalar_tensor_tensor(
                    out=tthr[0:1, lo:hi], in0=a_sx, scalar=0.5 / L,
                    op0=op.mult, in1=tmean[0:1, lo:hi], op1=op.add)
                # broadcast thr across partitions via PE (gpsimd is busy
                # with the collectives; a ones-matmul replicates the row)
                nc.tensor.matmul(
                    out=psum_tbc[:, lo:hi], lhsT=ones_row,
                    rhs=tthr[0:1, lo:hi], start=True, stop=True)
                nc.vector.tensor_scalar(
                    out=tbc[:, lo:hi], in0=psum_tbc[:, lo:hi], scalar1=1.0,
                    scalar2=None, op0=op.mult)

                notv3 = ntile.rearrange("p (r n) -> p r n", r=G)
                h4g = h4[:, :, lo:hi, :]
                # per-row notv rides the strip weak-count on the accumulator
                with nc.allow_low_precision(reason="0/1 mask, exact fp16"):
                    for r in range(lo, hi):
                        nc.vector.scalar_tensor_tensor(
                            out=notv3[:, r - lo], in0=B4mv[:, r], scalar=0.0,
                            op0=op.bypass,
                            in1=tbc[:, r:r + 1].broadcast_to([P, NB]),
                            op1=op.is_le,
                            accum_out=summ_sb[:, 30 + r:31 + r])
                    # mask weak/no-peak blocks to the sentinel on both planes
                    nc.vector.scalar_tensor_tensor(
                        out=h4g[:, 0], in0=notv3, scalar=BIG_H, op0=op.mult,
                        in1=h4g[:, 0], op1=op.add)
                    nc.vector.scalar_tensor_tensor(
                        out=h4g[:, 1], in0=notv3, scalar=BIG_H, op0=op.mult,
                        in1=h4g[:, 1], op1=op.add)

                # merge down to SEGW segments per strip; the host stitches the
                # rest (below ~16-wide the per-level ops are pure overhead)
                cur = h4g
                w = NB
                off = 0
                bufs_cycle = [treeB4, treeC4]
                lvl = 0
                while w > SEGW:
                    w2 = w // 2
                    out_h = bufs_cycle[lvl % 2][:, :, lo:hi, 0:w2]
                    with nc.allow_low_precision(reason="fp16 min"):
                        nc.vector.tensor_tensor(
                            out=out_h, in0=cur[:, :, :, 0:w:2],
                            in1=cur[:, :, :, 1:w:2], op=op.min)
                    g = gap3[:, lo:hi, 0:w2]
                    nc.vector.tensor_tensor(
                        out=g, in0=cur[:, 0, :, 1:w:2],
                        in1=cur[:, 1, :, 0:w:2], op=op.add)
                    nc.vector.reciprocal_approx_fast(
                        out=scr3[:, lo:hi, off:off + w2], in_=g)
                    off += w2
                    cur = out_h
                    w = w2
                    lvl += 1

                nc.vector.tensor_reduce(
                    out=summ_sb[:, 20 + lo:20 + hi], in_=scr3[:, lo:hi, 0:off],
                    axis=mybir.AxisListType.X, op=op.add)
                # stage this group's summaries contiguously for the DMA
                with nc.allow_low_precision(reason="fp16 position copy"):
                    nc.vector.tensor_scalar(
                        out=stg.rearrange("p (a r s) -> p a r s", a=2, r=R)
                        [:, :, lo:hi, :],
                        in0=cur, scalar1=1.0, scalar2=None, op0=op.mult)
                nc.sync.dma_start(
                    summ.ap()[:, 20:40].rearrange("p (a r) -> p a r", a=2)
                    [:, :, lo:hi],
                    summ_sb[:, 20:40].rearrange("p (a r) -> p a r", a=2)
                    [:, :, lo:hi])
            nc.sync.dma_start(summ2.ap(), stg)
            phase2.__exit__(None, None, None)

    nc.compile()
    return nc


def _get_runner():
    """Build once; return fn(in_maps) -> list of per-core {name: np.ndarray}."""
    if "runner" in _STATE:
        return _STATE["runner"]

    import jax
    from jax.sharding import Mesh, PartitionSpec
    from jax.experimental.shard_map import shard_map
    from concourse import bass2jax, mybir

    nc = _build_program()
    bass2jax.install_neuronx_cc_hook()

    partition_name = (
        nc.partition_id_tensor.name if nc.partition_id_tensor else None
    )
    in_names, out_names, out_avals, zero_outs = [], [], [], []
    for alloc in nc.m.functions[0].allocations:
        if not isinstance(alloc, mybir.MemoryLocationSet):
            continue
        name = alloc.memorylocations[0].name
        if alloc.kind == "ExternalInput":
            if name != partition_name:
                in_names.append(name)
        elif alloc.kind == "ExternalOutput":
            out_names.append(name)
            shape = tuple(alloc.tensor_shape)
            dtype = mybir.dt.np(alloc.dtype)
            out_avals.append(jax.core.ShapedArray(shape, dtype))
            zero_outs.append(np.zeros(shape, dtype))
    n_params = len(in_names)
    n_outs = len(out_avals)
    all_names = in_names + out_names
    if partition_name is not None:
        all_names = all_names + [partition_name]

    def _body(*args):
        operands = list(args)
        if partition_name is not None:
            operands.append(bass2jax.partition_id_tensor())
        outs = bass2jax._bass_exec_p.bind(
            *operands,
            out_avals=tuple(out_avals),
            in_names=tuple(all_names),
            out_names=tuple(out_names),
            lowering_input_output_aliases=(),
            sim_require_finite=False,
            sim_require_nnan=False,
            nc=nc,
        )
        return tuple(outs)

    devices = jax.devices()[:C]
    assert len(devices) == C, f"need {C} devices, have {len(jax.devices())}"
    mesh = Mesh(np.asarray(devices), ("core",))
    donate = tuple(range(n_params, n_params + n_outs))
    sharded = jax.jit(
        shard_map(
            _body, mesh=mesh,
            in_specs=(PartitionSpec("core"),) * (n_params + n_outs),
            out_specs=(PartitionSpec("core"),) * n_outs,
            check_rep=False,
        ),
        donate_argnums=donate,
        keep_unused=True,
    )

    def run(in_maps):
        concat_in = [
            np.concatenate([np.asarray(m[nm]) for m in in_maps], axis=0)
            for nm in in_names
        ]
        concat_zeros = [
            np.zeros((C * z.shape[0], *z.shape[1:]), z.dtype) for z in zero_outs
        ]
        out_arrs = sharded(*concat_in, *concat_zeros)
        return [
            {nm: np.asarray(out_arrs[i]).reshape(C, *out_avals[i].shape)[c]
             for i, nm in enumerate(out_names)}
            for c in range(C)
        ]

    run.in_names = in_names
    run.out_names = out_names
    run.sharded = sharded
    run.zero_outs = zero_outs
    run.nc = nc
    _STATE["runner"] = run
    return run


def make_in_maps(rppg, ppg):
    sigs = np.concatenate(
        [np.asarray(ppg, np.float32).reshape(N, L),
         np.asarray(rppg, np.float32).reshape(N, L)], axis=0)
    padded = np.full((R, L + 10), -np.inf, np.float32)
    padded[:, 5:5 + L] = sigs
    win = np.lib.stride_tricks.sliding_window_view(padded, TILE_W, axis=1)
    in_maps = []
    for c in range(C):
        xin_c = np.ascontiguousarray(win[:, c * SEG:c * SEG + SEG:PW, :])
        in_maps.append({"xin": xin_c})
    return in_maps


def stitch(results, fs):
    summ = np.stack([results[c]["summ"] for c in range(C)])  # [C, 128, 40]
    # [C, 128, 2, R, SEGW] first/last peak position per 128-sample segment
    s2 = np.stack([results[c]["summ2"] for c in range(C)]).reshape(
        C, P, 2, R, SEGW)
    # positions are strip-local; add per-(core, partition) global offsets
    offs = np.broadcast_to(
        (np.arange(C)[:, None] * SEG + np.arange(P)[None, :] * PW)
        .astype(np.float64)[:, :, None], (C, P, SEGW)).reshape(-1)
    hr = np.zeros(R)
    for r in range(R):
        f = s2[:, :, 0, r, :].reshape(-1).astype(np.float64)
        g = -s2[:, :, 1, r, :].reshape(-1).astype(np.float64)
        s = summ[:, :, 2 * R + r].astype(np.float64).sum()
        n = (512.0 - summ[:, :, 3 * R + r].astype(np.float64)).sum()
        ne = f < VALID_LT
        fs_, gs_ = f[ne] + offs[ne], g[ne] + offs[ne]
        s += (1.0 / (fs_[1:] - gs_[:-1])).sum()
        hr[r] = 60.0 * float(fs) * s / (n - 1.0)
    return np.float32(np.mean(np.abs(hr[0:N] - hr[N:R]) / hr[0:N]))


def kernel(rppg, ppg, fs, epoch):
    run = _get_runner()
    results = run(make_in_maps(rppg, ppg))
    return stitch(results, fs)


# revision 53
# speedup vs baseline: 1.0169x; 1.0169x over previous
"""PeakDetectionLoss on 8 Trainium2 cores.

Sharding: time axis split into 8 segments (one per core), all 10 signal rows
(5 ppg + 5 rppg) on every core. Host pre-pads 5 samples of -inf at the global
edges and hands each core overlapping [128, 2058] windows per row, so the
width-11 sliding max needs no device halo exchange.

Per-row stats (peak count, signal sum, peak-value sum) are reduced across
partitions with a ones-matmul and across cores with two tiny AllReduces: the
one for rows 0-5 is issued mid-row-loop so its ~35us ring latency hides under
rows 6-9's compute, and the late one only gates the phase-2 work for rows
6-9. Slot/position extraction is deferred until after the collectives are
issued so it fills the remaining latency. Positions are strip-local fp16
(host adds global offsets; 24576 is the fp16-exact "no peak" sentinel, whose
reciprocal contribution to the gap sums is ~2e-5 per term and cancels between
the ppg/rppg rows). Row transfers are split across both HWDGE queues (one
sustains only ~75 GB/s), the ar DMAs ride the gpsimd SWDGE queue, and the
threshold partition-broadcast is a PE ones-matmul, so none of them
head-of-line-block the row DMA streams or the DVE. Host stitches the 1024
strip summaries per row (first/last peak, sum of 1/gap, weak count).
"""
import os
import sys

for _p in ("/opt/trn_rl_repo", "/root/.axon_site/_ro/trn_rl_repo"):
    if _p not in sys.path:
        sys.path.append(_p)

import numpy as np

N = 5
L = 2097152
C = 8
SEG = L // C            # 262144
P = 128
PW = SEG // P           # 2048
NB = PW // 4            # 512
TILE_W = PW + 10        # 2058
R = 2 * N               # 10 rows per core
BIG_H = 24576.0         # fp16-exact sentinel; 2*BIG_H stays finite in fp16
VALID_LT = 16384.0      # host-side validity threshold for positions
SEGW = 16               # segments per strip shipped to the host stitcher

_STATE = {}


def _build_program():
    from concourse import bacc, tile, mybir
    from concourse.alu_op_type import AluOpType as op

    f32 = mybir.dt.float32
    f16 = mybir.dt.float16
    i16 = mybir.dt.int16
    act = mybir.ActivationFunctionType
    nc = bacc.Bacc("TRN2", target_bir_lowering=False, debug=False, num_devices=C)

    xin = nc.dram_tensor("xin", [R, P, TILE_W], f32, kind="ExternalInput")
    summ = nc.dram_tensor("summ", [P, 40], f32, kind="ExternalOutput")
    summ2 = nc.dram_tensor("summ2", [P, 2 * R * SEGW], f16,
                           kind="ExternalOutput")

    with tile.TileContext(nc) as tc:
        with (
            tc.tile_pool(name="sb", bufs=1) as sb,
            tc.tile_pool(name="dram", bufs=1, space="DRAM") as dram,
            tc.tile_pool(name="ps", bufs=1, space="PSUM") as ps,
        ):
            ones = sb.tile([P, 1], f32, tag="ones")
            ones_row = sb.tile([1, P], f32, tag="ones_row")
            # split stats tiles so the early AllReduce's matmul doesn't
            # falsely depend on rows 7-9's accumulates
            statsA = sb.tile([P, 3 * 6], f32, tag="statsA")
            statsB = sb.tile([P, 3 * (R - 6)], f32, tag="statsB")
            # bn (peak value per block) shares storage with the recip scratch
            # (bn is dead once B4m is built; recips are written later)
            bnscr = sb.tile([P, R * NB], f32, tag="bnscr")
            m1a = sb.tile([P, R * PW], f16, tag="m1a")
            n1a = sb.tile([P, R * NB], f16, tag="n1a")
            p1a = sb.tile([P, R * NB], f16, tag="p1a")
            p2a = sb.tile([P, R * NB], f16, tag="p2a")
            h = sb.tile([P, 2 * R * NB], f16, tag="h")
            # B4m holds the amplitude-test values; its first 2048 slots double
            # as the dummy activation output for the Sx accumulate
            B4m = sb.tile([P, R * NB], f32, tag="B4m")
            treeB = sb.tile([P, 2 * R * 256], f16, tag="treeB")
            treeC = sb.tile([P, 2 * R * 128], f16, tag="treeC")
            notvB = sb.tile([P, (R - 6) * NB], f16, tag="notvB")
            gapb = sb.tile([P, R * 256], f32, tag="gapb")
            r4 = sb.tile([P, NB], f16, tag="r4")
            tbc = sb.tile([P, R], f32, tag="tbc")
            arsbA = sb.tile([1, 3 * 6], f32, tag="arsbA")
            arsbB = sb.tile([1, 3 * (R - 6)], f32, tag="arsbB")
            arstA = sb.tile([1, 3 * 6], f32, tag="arstA")
            arstB = sb.tile([1, 3 * (R - 6)], f32, tag="arstB")
            trec = sb.tile([1, R], f32, tag="trec")
            tmean = sb.tile([1, R], f32, tag="tmean")
            tthr = sb.tile([1, R], f32, tag="tthr")
            summ_sb = sb.tile([P, 40], f32, tag="summ_sb")
            stg = sb.tile([P, 2 * R * SEGW], f16, tag="stg")
            # per-row chain scratch (statically reused; M8 aliases M2)
            M2 = sb.tile([P, 2057], f32, tag="M2")
            M4 = sb.tile([P, 2055], f32, tag="M4")
            Wt = sb.tile([P, PW], f32, tag="Wt")

            SPLIT = 18  # stats columns 0:18 (rows 0-5) reduce early
            ar_inA = dram.tile([1, SPLIT], f32)
            ar_outA = dram.tile([1, SPLIT], f32)
            ar_inB = dram.tile([1, 3 * R - SPLIT], f32)
            ar_outB = dram.tile([1, 3 * R - SPLIT], f32)
            psum_A = ps.tile([1, SPLIT], f32)
            psum_B = ps.tile([1, 3 * R - SPLIT], f32)
            psum_tbc = ps.tile([P, R], f32)

            xin_ap = xin.ap()
            nc.vector.memset(ones, 1.0)
            nc.vector.memset(ones_row, 1.0)
            nc.gpsimd.iota(r4, pattern=[[4, NB]], base=0, channel_multiplier=0,
                           allow_small_or_imprecise_dtypes=True)

            aBv = gapb.bitcast(f16)  # [P, 2*R*NB] fp16 view of the gap buffer
            m1v = m1a.rearrange("p (r n) -> p r n", r=R)
            n1v = n1a.rearrange("p (r n) -> p r n", r=R)
            p1v = p1a.rearrange("p (r n) -> p r n", r=R)
            p2v = p2a.rearrange("p (r n) -> p r n", r=R)
            bnv = bnscr.rearrange("p (r n) -> p r n", r=R)
            B4mv = B4m.rearrange("p (r n) -> p r n", r=R)

            for r in range(R):
                xt = sb.tile([P, TILE_W], f32, tag="xt", bufs=2, name=f"xt{r}")
                # each row's transfer split across both HWDGE queues
                # (one queue sustains only ~75 GB/s)
                nc.sync.dma_start(xt[:, 0:1029], xin_ap[r][:, 0:1029])
                nc.scalar.dma_start(xt[:, 1029:TILE_W], xin_ap[r][:, 1029:TILE_W])

                # sliding max chain (window 11, centered at xt[:, j+5])
                nc.vector.tensor_tensor(
                    out=M2, in0=xt[:, 0:2057], in1=xt[:, 1:2058], op=op.max)
                nc.vector.tensor_tensor(
                    out=M4, in0=M2[:, 0:2055], in1=M2[:, 2:2057], op=op.max)
                M8 = M2[:, 0:2051]  # M2 storage reused for M8
                nc.vector.tensor_tensor(
                    out=M8, in0=M4[:, 0:2051], in1=M4[:, 4:2055], op=op.max)
                nc.vector.tensor_tensor(
                    out=Wt, in0=M8[:, 0:PW], in1=M2[:, 3:3 + PW], op=op.max)

                stt = statsA if r < 6 else statsB
                sc = 3 * (r if r < 6 else r - 6)
                xc = xt[:, 5:5 + PW]
                # m1 = (x == window max); accum -> per-partition peak count
                nc.vector.scalar_tensor_tensor(
                    out=m1v[:, r], in0=xc, scalar=0.0, op0=op.bypass,
                    in1=Wt, op1=op.is_ge,
                    accum_out=stt[:, sc:sc + 1])
                # Sx on ScalarE (free accumulate; dummy out over B4m scratch)
                nc.scalar.activation(
                    out=B4m[:, 0:PW], in_=xc, func=act.Copy,
                    accum_out=stt[:, sc + 1:sc + 2])
                # peaks per aligned 4-block (0 or 1 each)
                with nc.allow_low_precision(reason="0/1 sums <=4, exact fp16"):
                    nc.vector.tensor_reduce(
                        out=n1v[:, r],
                        in_=m1v[:, r].rearrange("p (b k) -> p b k", k=4),
                        axis=mybir.AxisListType.X, op=op.add)
                B4 = M4[:, 5:2052:4]  # aligned block-4 max, free view of M4
                # bn = B4*n1 (fused accum -> sum of peak values)
                nc.vector.scalar_tensor_tensor(
                    out=bnv[:, r], in0=B4, scalar=0.0, op0=op.bypass,
                    in1=n1v[:, r], op1=op.mult,
                    accum_out=stt[:, sc + 2:sc + 3])

                if r == 5:
                    # aB-G1 on the idle Act engine while rows 6-9 compute
                    nc.scalar.activation(
                        out=aBv[:, 0:6 * NB], in_=n1a[:, 0:6 * NB],
                        func=act.Copy, bias=-BIG_H, scale=BIG_H)
                    # early AllReduce for rows 0-5; its ~35us ring latency
                    # hides under rows 6-9's compute. The psum->sbuf copy
                    # runs on DVE and the ar DMAs on the gpsimd SWDGE queue
                    # so neither blocks the xt row-DMA streams.
                    nc.tensor.matmul(
                        out=psum_A[0:1, :], lhsT=ones, rhs=statsA,
                        start=True, stop=True)
                    nc.vector.tensor_scalar(
                        out=arstA, in0=psum_A[0:1, :], scalar1=1.0,
                        scalar2=None, op0=op.mult)
                    nc.gpsimd.dma_start(ar_inA, arstA)
                    nc.gpsimd.collective_compute(
                        "AllReduce", op.add, replica_groups=[list(range(C))],
                        ins=[ar_inA.opt()], outs=[ar_outA.opt()])
                    nc.gpsimd.dma_start(arsbA, ar_outA)

            # deferred slot/position work overlaps the collective
            with nc.allow_low_precision(reason="slot sums <=3, exact fp16"):
                nc.vector.scalar_tensor_tensor(
                    out=p1v, in0=m1v[:, :, 2:PW:4], scalar=2.0, op0=op.mult,
                    in1=m1v[:, :, 1:PW:4], op1=op.add)
                nc.vector.scalar_tensor_tensor(
                    out=p2v, in0=m1v[:, :, 3:PW:4], scalar=3.0, op0=op.mult,
                    in1=p1v, op1=op.add)
            hpos = h[:, 0:R * NB]
            hneg = h[:, R * NB:2 * R * NB]
            with nc.allow_low_precision(reason="positions <=2047, exact fp16"):
                nc.vector.tensor_tensor(
                    out=hpos.rearrange("p (r n) -> p r n", r=R), in0=p2v,
                    in1=r4.unsqueeze(1).broadcast_to([P, R, NB]), op=op.add)
            nc.scalar.mul(hneg, hpos, -1.0)
            # aB-G2 = 0 if peak else -BIG_H (G1's ran mid-row-loop);
            # B4m = bn + aB per group (aB lives in gapb, dead before gaps)
            nc.scalar.activation(out=aBv[:, 6 * NB:R * NB],
                                 in_=n1a[:, 6 * NB:R * NB], func=act.Copy,
                                 bias=-BIG_H, scale=BIG_H)
            nc.vector.tensor_tensor(
                out=B4m[:, 0:6 * NB], in0=bnscr[:, 0:6 * NB],
                in1=aBv[:, 0:6 * NB], op=op.add)
            nc.vector.tensor_tensor(
                out=B4m[:, 6 * NB:R * NB], in0=bnscr[:, 6 * NB:R * NB],
                in1=aBv[:, 6 * NB:R * NB], op=op.add)

            # late AllReduce for rows 6-9, issued after the deferred DVE work
            # so its psum->sbuf copy can't head-of-line-block that work
            nc.tensor.matmul(
                out=psum_B[0:1, :], lhsT=ones, rhs=statsB,
                start=True, stop=True)
            nc.vector.tensor_scalar(
                out=arstB, in0=psum_B[0:1, :], scalar1=1.0,
                scalar2=None, op0=op.mult)
            nc.gpsimd.dma_start(ar_inB, arstB)
            nc.gpsimd.collective_compute(
                "AllReduce", op.add, replica_groups=[list(range(C))],
                ins=[ar_inB.opt()], outs=[ar_outB.opt()])
            nc.gpsimd.dma_start(arsbB, ar_outB)

            # phase 2 runs in two row groups: rows 0-6 only need the early
            # AllReduce, rows 7-9 wait for the late one
            h4 = h.rearrange("p (a r n) -> p a r n", a=2, r=R)
            treeB4 = treeB.rearrange("p (a r c) -> p a r c", a=2, r=R)
            treeC4 = treeC.rearrange("p (a r c) -> p a r c", a=2, r=R)
            gap3 = gapb.rearrange("p (r c) -> p r c", r=R)
            scr3 = bnscr.rearrange("p (r c) -> p r c", r=R)

            # G1's notv borrows treeB's first rows (dead before L1 lands there)
            # Low scheduler priority (negative offset = later) keeps these
            # collective-dependent ops from being hoisted into the row loop's
            # DVE stream, where their arsb wait would block it head-of-line.
            phase2 = tc.high_priority(offset=-1000000)
            phase2.__enter__()
            # WAW gate: the static scheduler underestimates the collective
            # latency and would slot the arsb-dependent threshold ops into
            # the DVE stream ahead of row 9, stalling it head-of-line. A
            # dummy write to trec that reads row 9's mask pins them after.
            nc.vector.tensor_scalar(
                out=trec[0:1, :], in0=m1a[0:1, 9 * PW:9 * PW + R],
                scalar1=0.0, scalar2=None, op0=op.mult)
            for lo, hi, ntile in ((0, 6, treeB[:, 0:6 * NB]), (6, R, notvB)):
                G = hi - lo
                arsb_g = arsbA if lo == 0 else arsbB
                # threshold t_r = Sx_r/(2L) + 0.5*sv_r/npk_r (raw-space)
                a_npk = arsb_g[0:1, 0:3 * G:3]
                a_sx = arsb_g[0:1, 1:3 * G:3]
                a_sv = arsb_g[0:1, 2:3 * G:3]
                nc.vector.reciprocal(out=trec[0:1, lo:hi], in_=a_npk)
                nc.vector.scalar_tensor_tensor(
                    out=tmean[0:1, lo:hi], in0=trec[0:1, lo:hi], scalar=0.5,
                    op0=op.mult, in1=a_sv, op1=op.mult)
                nc.vector.scalar_tensor_tensor(
                    out=tthr[0:1, lo:hi], in0=a_sx, scalar=0.5 / L,
                    op0=op.mult, in1=tmean[0:1, lo:hi], op1=op.add)
                # broadcast thr across partitions via PE (gpsimd is busy
                # with the collectives; a ones-matmul replicates the row)
                nc.tensor.matmul(
                    out=psum_tbc[:, lo:hi], lhsT=ones_row,
                    rhs=tthr[0:1, lo:hi], start=True, stop=True)
                nc.vector.tensor_scalar(
                    out=tbc[:, lo:hi], in0=psum_tbc[:, lo:hi], scalar1=1.0,
                    scalar2=None, op0=op.mult)

                notv3 = ntile.rearrange("p (r n) -> p r n", r=G)
                h4g = h4[:, :, lo:hi, :]
                # per-row notv rides the strip weak-count on the accumulator
                with nc.allow_low_precision(reason="0/1 mask, exact fp16"):
                    for r in range(lo, hi):
                        nc.vector.scalar_tensor_tensor(
                            out=notv3[:, r - lo], in0=B4mv[:, r], scalar=0.0,
                            op0=op.bypass,
                            in1=tbc[:, r:r + 1].broadcast_to([P, NB]),
                            op1=op.is_le,
                            accum_out=summ_sb[:, 30 + r:31 + r])
                    # mask weak/no-peak blocks to the sentinel on both planes
                    nc.vector.scalar_tensor_tensor(
                        out=h4g[:, 0], in0=notv3, scalar=BIG_H, op0=op.mult,
                        in1=h4g[:, 0], op1=op.add)
                    nc.vector.scalar_tensor_tensor(
                        out=h4g[:, 1], in0=notv3, scalar=BIG_H, op0=op.mult,
                        in1=h4g[:, 1], op1=op.add)

                # merge down to SEGW segments per strip; the host stitches the
                # rest (below ~16-wide the per-level ops are pure overhead)
                cur = h4g
                w = NB
                off = 0
                bufs_cycle = [treeB4, treeC4]
                lvl = 0
                while w > SEGW:
                    w2 = w // 2
                    out_h = bufs_cycle[lvl % 2][:, :, lo:hi, 0:w2]
                    with nc.allow_low_precision(reason="fp16 min"):
                        nc.vector.tensor_tensor(
                            out=out_h, in0=cur[:, :, :, 0:w:2],
                            in1=cur[:, :, :, 1:w:2], op=op.min)
                    g = gap3[:, lo:hi, 0:w2]
                    nc.vector.tensor_tensor(
                        out=g, in0=cur[:, 0, :, 1:w:2],
                        in1=cur[:, 1, :, 0:w:2], op=op.add)
                    nc.vector.reciprocal_approx_fast(
                        out=scr3[:, lo:hi, off:off + w2], in_=g)
                    off += w2
                    cur = out_h
                    w = w2
                    lvl += 1

                nc.vector.tensor_reduce(
                    out=summ_sb[:, 20 + lo:20 + hi], in_=scr3[:, lo:hi, 0:off],
                    axis=mybir.AxisListType.X, op=op.add)
                # stage this group's summaries contiguously for the DMA
                # (on the half-idle Act engine, off the saturated DVE)
                nc.scalar.activation(
                    out=stg.rearrange("p (a r s) -> p a r s", a=2, r=R)
                    [:, :, lo:hi, :],
                    in_=cur, func=act.Copy)
                nc.sync.dma_start(
                    summ.ap()[:, 20:40].rearrange("p (a r) -> p a r", a=2)
                    [:, :, lo:hi],
                    summ_sb[:, 20:40].rearrange("p (a r) -> p a r", a=2)
                    [:, :, lo:hi])
            nc.sync.dma_start(summ2.ap(), stg)
            phase2.__exit__(None, None, None)

    nc.compile()
    return nc


def _get_runner():
    """Build once; return fn(in_maps) -> list of per-core {name: np.ndarray}."""
    if "runner" in _STATE:
        return _STATE["runner"]

    import jax
    from jax.sharding import Mesh, PartitionSpec
    from jax.experimental.shard_map import shard_map
    from concourse import bass2jax, mybir

    nc = _build_program()
    bass2jax.install_neuronx_cc_hook()

    partition_name = (
        nc.partition_id_tensor.name if nc.partition_id_tensor else None
    )
    in_names, out_names, out_avals, zero_outs = [], [], [], []
    for alloc in nc.m.functions[0].allocations:
        if not isinstance(alloc, mybir.MemoryLocationSet):
            continue
        name = alloc.memorylocations[0].name
        if alloc.kind == "ExternalInput":
            if name != partition_name:
                in_names.append(name)
        elif alloc.kind == "ExternalOutput":
            out_names.append(name)
            shape = tuple(alloc.tensor_shape)
            dtype = mybir.dt.np(alloc.dtype)
            out_avals.append(jax.core.ShapedArray(shape, dtype))
            zero_outs.append(np.zeros(shape, dtype))
    n_params = len(in_names)
    n_outs = len(out_avals)
    all_names = in_names + out_names
    if partition_name is not None:
        all_names = all_names + [partition_name]

    def _body(*args):
        operands = list(args)
        if partition_name is not None:
            operands.append(bass2jax.partition_id_tensor())
        outs = bass2jax._bass_exec_p.bind(
            *operands,
            out_avals=tuple(out_avals),
            in_names=tuple(all_names),
            out_names=tuple(out_names),
            lowering_input_output_aliases=(),
            sim_require_finite=False,
            sim_require_nnan=False,
            nc=nc,
        )
        return tuple(outs)

    devices = jax.devices()[:C]
    assert len(devices) == C, f"need {C} devices, have {len(jax.devices())}"
    mesh = Mesh(np.asarray(devices), ("core",))
    donate = tuple(range(n_params, n_params + n_outs))
    sharded = jax.jit(
        shard_map(
            _body, mesh=mesh,
            in_specs=(PartitionSpec("core"),) * (n_params + n_outs),
            out_specs=(PartitionSpec("core"),) * n_outs,
            check_rep=False,
        ),
        donate_argnums=donate,
        keep_unused=True,
    )

    def run(in_maps):
        concat_in = [
            np.concatenate([np.asarray(m[nm]) for m in in_maps], axis=0)
            for nm in in_names
        ]
        concat_zeros = [
            np.zeros((C * z.shape[0], *z.shape[1:]), z.dtype) for z in zero_outs
        ]
        out_arrs = sharded(*concat_in, *concat_zeros)
        return [
            {nm: np.asarray(out_arrs[i]).reshape(C, *out_avals[i].shape)[c]
             for i, nm in enumerate(out_names)}
            for c in range(C)
        ]

    run.in_names = in_names
    run.out_names = out_names
    run.sharded = sharded
    run.zero_outs = zero_outs
    run.nc = nc
    _STATE["runner"] = run
    return run


def make_in_maps(rppg, ppg):
    sigs = np.concatenate(
        [np.asarray(ppg, np.float32).reshape(N, L),
         np.asarray(rppg, np.float32).reshape(N, L)], axis=0)
    padded = np.full((R, L + 10), -np.inf, np.float32)
    padded[:, 5:5 + L] = sigs
    win = np.lib.stride_tricks.sliding_window_view(padded, TILE_W, axis=1)
    in_maps = []
    for c in range(C):
        xin_c = np.ascontiguousarray(win[:, c * SEG:c * SEG + SEG:PW, :])
        in_maps.append({"xin": xin_c})
    return in_maps


def stitch(results, fs):
    summ = np.stack([results[c]["summ"] for c in range(C)])  # [C, 128, 40]
    # [C, 128, 2, R, SEGW] first/last peak position per 128-sample segment
    s2 = np.stack([results[c]["summ2"] for c in range(C)]).reshape(
        C, P, 2, R, SEGW)
    # positions are strip-local; add per-(core, partition) global offsets
    offs = np.broadcast_to(
        (np.arange(C)[:, None] * SEG + np.arange(P)[None, :] * PW)
        .astype(np.float64)[:, :, None], (C, P, SEGW)).reshape(-1)
    hr = np.zeros(R)
    for r in range(R):
        f = s2[:, :, 0, r, :].reshape(-1).astype(np.float64)
        g = -s2[:, :, 1, r, :].reshape(-1).astype(np.float64)
        s = summ[:, :, 2 * R + r].astype(np.float64).sum()
        n = (512.0 - summ[:, :, 3 * R + r].astype(np.float64)).sum()
        ne = f < VALID_LT
        fs_, gs_ = f[ne] + offs[ne], g[ne] + offs[ne]
        s += (1.0 / (fs_[1:] - gs_[:-1])).sum()
        hr[r] = 60.0 * float(fs) * s / (n - 1.0)
    return np.float32(np.mean(np.abs(hr[0:N] - hr[N:R]) / hr[0:N]))


def kernel(rppg, ppg, fs, epoch):
    run = _get_runner()
    results = run(make_in_maps(rppg, ppg))
    return stitch(results, fs)
